# revision 27
# baseline (speedup 1.0000x reference)
"""Trainium2 Bass kernel for BaseBidirectionalAttention (fused-linear version).

Problem shapes (hardcoded): B=32, C=1024, Q=128, D=256, F=4D=1024.
Sharding: data-parallel over batch across 8 cores (4 batch elems/core);
weights replicated.

Algebraic restructurings vs the reference (all exact in real arithmetic):
  1. Fused linears: masking is row-wise and there is no nonlinearity between
     the two linears, so
       relu(((att@W1.T+b1)*m @ W2.T + b2)*m) = relu((att@W12.T + b12)*m)
     with W12 = W2@W1, b12 = W2@b1 + b2 precomputed on host.  Halves the
     dominant matmul work.
  2. att = [ctx, c2q, ctx*c2q, ctx*q2c]; q2c is constant over context rows,
     so the ctx and ctx*q2c pieces merge via a per-elem scaled weight block
     W_eff = A + D*diag(q2c)  (Pool-engine prep, no extra matmul k-steps).
  3. c2q = P @ question (P = softmax(sim) over q), so
     c2q @ B.T = P @ (question @ B.T) = P @ QB  -- and QB' = QB + b12 is
     input-only, precomputed on HOST (softmax rows sum to 1, so the bias
     rides along the P matmul for free).
  4. cwc = ctx.w_context folds into the sim matmul as a 129th moving column;
     qwq = question.w_question folds in as a K=1 accumulating matmul row.
  5. All input-only layout work (ctx/question transposes, fp16 casts,
     q*wm scaling, qwq row, QB') happens on host in _host_prep -- the device
     spends its cycles only on data-dependent compute.

Per-core per-elem device program (natural-layout output):
  sim(C,129)  = ctxT16.T @ [q*wm | w_c]  (+ qwq via K=1 row)      PE fp16
  P(C,Q)      = softmax_q(sim[:, :128])                           DVE/ACT
  PT(Q,C) transposes, cxc=(ctx*c2q)^T, W_eff=A+D*diag(q2c)
  out(C,F)    = relu((ctx@W_eff.T + cxc.T@C.T + P@QB') * m)       PE fp16

Everything heavy runs fp16 on the PE (1 cyc/row, hideable LDWEIGHTS+FWL);
softmax statistics, exp, q2c weighting and PSUM accumulation stay fp32.
Output is stored fp16 (halves the dominant DMA stream), upcast on host.
The per-elem fused layer is software-pipelined one stage behind the
attention phase: its matmuls are emitted interleaved into the next elem's
attention so PE never idles during the serial softmax/q2c chains.
"""

import sys

if "/opt/trn_rl_repo" not in sys.path:
    sys.path.insert(0, "/opt/trn_rl_repo")

import numpy as np

import concourse.bass as bass
import concourse.mybir as mybir
import concourse.tile as tile
from concourse import bacc
from concourse.bass_utils import run_bass_kernel_spmd
from concourse.masks import make_identity

B, C, Q, D = 32, 1024, 128, 256
F = 4 * D
NCORES = 8
BPC = B // NCORES  # batch elems per core
P = 128
CT = C // P   # 8 c-tiles
FT = F // P   # 8 f-tiles
DH = D // P   # 2 halves of D
QP = Q + 8    # padded moving-dim for the sim rhs (alignment)

FP32 = mybir.dt.float32
FP16 = mybir.dt.float16
AX = mybir.AxisListType.X
AF = mybir.ActivationFunctionType


def _build_body(es, tc, outs, ins, n_elems=BPC, reps=1):
    nc = tc.nc
    ctxT_d, cnat_d, qst_d, qmx_d, qwq_d, qb_d, w12t_d, mT_d = ins
    out_d = outs[0]

    const = es.enter_context(tc.tile_pool(name="const", bufs=1))
    weights = es.enter_context(tc.tile_pool(name="weights", bufs=1))
    loads = es.enter_context(tc.tile_pool(name="loads", bufs=3))
    work = es.enter_context(tc.tile_pool(name="work", bufs=1))
    outp = es.enter_context(tc.tile_pool(name="outp", bufs=4))
    psA = es.enter_context(tc.tile_pool(name="psA", bufs=5, space="PSUM"))
    psB = es.enter_context(tc.tile_pool(name="psB", bufs=3, space="PSUM"))

    # ---- constants / replicated weights ----
    ident = const.tile([P, P], FP32)
    make_identity(nc, ident)
    ident16 = const.tile([P, P], FP16)
    make_identity(nc, ident16)
    ones_row = const.tile([1, P], FP32)
    nc.vector.memset(ones_row, 1.0)
    ones16 = const.tile([1, P], FP16)
    nc.vector.memset(ones16, 1.0)

    def load_elem(b, idx):
        ctxT16 = loads.tile([P, DH, C], FP16, tag="ctxT16", name=f"ctxT{idx}")
        src = ctxT_d[b].rearrange("(h p) c -> p h c", p=P)
        nc.sync.dma_start(ctxT16[:, 0, :], src[:, 0])
        nc.sync.dma_start(ctxT16[:, 1, :], src[:, 1])
        cnat16 = loads.tile([P, CT, D + 4], FP16, tag="cnat16",
                            name=f"cnat{idx}")
        csrc = cnat_d[b].rearrange("(t p) d -> p t d", p=P)
        half = CT // 2
        nc.sync.dma_start(cnat16[:, :half, 0:D + 1], csrc[:, :half])
        nc.sync.dma_start(cnat16[:, half:, 0:D + 1], csrc[:, half:])
        qst16 = loads.tile([P, D], FP16, tag="qst16", name=f"qst{idx}")
        nc.sync.dma_start(qst16[:], qst_d[b])
        qmx16 = loads.tile([P, DH, QP], FP16, tag="qmx16", name=f"qmx{idx}")
        nc.sync.dma_start(qmx16[:], qmx_d[b].rearrange("(h p) j -> p h j", p=P))
        qwqx16 = loads.tile([1, QP], FP16, tag="qwqx16", name=f"qwq{idx}")
        nc.sync.dma_start(qwqx16[:], qwq_d[b])
        QB16 = loads.tile([P, F], FP16, tag="QB16", name=f"QB{idx}")
        nc.sync.dma_start(QB16[:], qb_d[b])
        return ctxT16, cnat16, qst16, qmx16, qwqx16, QB16

    # elem-0 loads go before the big weight DMA (single-shot only: with a
    # For_i timing loop the hoisted tile's slot would be recycled in-loop)
    pend = load_elem(0, 0) if reps == 1 else None

    w12t16 = weights.tile([P, FT, F], FP16)  # [fl, k, f'] = W12[f', k*128+fl]
    nc.sync.dma_start(w12t16[:], w12t_d.rearrange("(k p) f -> p k f", p=P))
    mT = const.tile([P, n_elems * CT], FP32)  # [p, b*8+t] = mask[b, t*128+p]
    nc.sync.dma_start(mT[:], mT_d)

    if reps > 1:
        es.enter_context(tc.For_i(0, reps, 1))

    def emit_fused(stage, cts):
        """Fused layer for `stage`'s elem (natural layout) + relu*mask +
        store.  Emitted interleaved into the NEXT elem's attention phase so
        its matmuls fill PE during that elem's serial q2c chain.  Piece
        order puts the late-arriving W_eff operands last."""
        if stage is None:
            return
        fb, fctxT16, fcxc16, fPT16, fQB16, fweff16 = stage
        for ct in cts:
            sl = slice(ct * P, (ct + 1) * P)
            pieces = [
                (fcxc16[:, 0, sl], w12t16[:, 4, :]),
                (fcxc16[:, 1, sl], w12t16[:, 5, :]),
                (fPT16[:, sl], fQB16[:]),
                (fctxT16[:, 0, sl], fweff16[:, 0, :]),
                (fctxT16[:, 1, sl], fweff16[:, 1, :]),
            ]
            p2 = [psA.tile([P, 512], FP32, tag="ps_mm", name=f"p2_{fb}{ct}{fh}")
                  for fh in range(2)]
            npc = len(pieces)
            for i, (lh, rh) in enumerate(pieces):
                for fh in range(2):
                    nc.tensor.matmul(
                        p2[fh][:], lh, rh[:, fh * 512:(fh + 1) * 512],
                        start=(i == 0), stop=(i == npc - 1),
                    )
            osb = outp.tile([P, F], FP16, tag="osb")
            mcol = mT[:, fb * CT + ct:fb * CT + ct + 1]
            # alternate evict engines so neither FIFO gates the PSUM ring
            nc.scalar.activation(osb[:, 0:512], p2[0][:], AF.Relu, scale=mcol)
            nc.vector.tensor_scalar(osb[:, 512:1024], p2[1][:], 0.0, mcol,
                                    op0=mybir.AluOpType.max,
                                    op1=mybir.AluOpType.mult)
            nc.sync.dma_start(out_d[fb, sl, :], osb[:])

    prev_stage = None
    for b in range(n_elems):
        # ---- loads (elem b prefetched; prefetch b+1 now) ----
        if pend is None:
            pend = load_elem(b, b)
        ctxT16, cnat16, qst16, qmx16, qwqx16, QB16 = pend
        pend = load_elem(b + 1, b + 1) if b + 1 < n_elems else None

        # ---- sim tiles + softmax over q (free dim), fused(b-1) groups
        # interleaved so PE never waits on the softmax consumers ----
        nmx = work.tile([P, CT], FP32, tag="nmx")    # negated row max
        sume = work.tile([P, CT], FP32, tag="sume")
        rs = work.tile([P, CT], FP32, tag="rs")
        pcwc = work.tile([P, CT], FP32, tag="pcwc")  # cwc columns [c_l, t]
        Pm16 = work.tile([P, CT, Q], FP16, tag="Pm16")

        def sim_tile(t):
            ps = psB.tile([P, Q + 1], FP32, tag="ps_small")
            for dh in range(DH):
                nc.tensor.matmul(
                    ps[:], ctxT16[:, dh, t * P:(t + 1) * P],
                    qmx16[:, dh, 0:Q + 1],
                    start=(dh == 0), stop=False,
                )
            nc.tensor.matmul(ps[:], ones16[:], qwqx16[0:1, 0:Q + 1],
                             start=False, stop=True)
            nc.vector.reduce_max(nmx[:, t:t + 1], ps[:, 0:Q], axis=AX,
                                 negate=True)
            nc.scalar.activation(
                Pm16[:, t, :], ps[:, 0:Q], AF.Exp, bias=nmx[:, t:t + 1],
                accum_out=sume[:, t:t + 1],
            )
            nc.vector.tensor_copy(pcwc[:, t:t + 1], ps[:, Q:Q + 1])
            nc.vector.reciprocal(rs[:, t:t + 1], sume[:, t:t + 1])
            nc.vector.tensor_scalar_mul(Pm16[:, t, :], Pm16[:, t, :],
                                        rs[:, t:t + 1])

        for t in range(4):
            sim_tile(t)
        emit_fused(prev_stage, [0])
        for t in range(4, CT):
            sim_tile(t)
        emit_fused(prev_stage, [1])

        # ---- q2c chain (DVE/ACT) with PT/c2q/fused as PE fill ----
        madj = work.tile([P, CT], FP32, tag="madj")  # m_c = cwc - nmx
        nc.vector.tensor_sub(madj[:], pcwc[:], nmx[:])
        colmin = work.tile([P, 1], FP32, tag="colmin")
        nc.vector.reduce_max(colmin[:], madj[:], axis=AX, negate=True)

        PT16 = work.tile([P, C], FP16, tag="PT16", bufs=2, name=f"PT16_{b}")
        cxc16 = work.tile([P, DH, C], FP16, tag="cxc16", bufs=2,
                          name=f"cxc16_{b}")
        for g in range(2):
            pt16 = psA.tile([P, 512], FP16, tag="ps_mm", name=f"pt16{g}")
            for j in range(4):
                t = g * 4 + j
                nc.tensor.transpose(pt16[:, j * P:(j + 1) * P], Pm16[:, t, :],
                                    ident16[:])
            nc.scalar.copy(PT16[:, g * 512:(g + 1) * 512], pt16[:])
        pcm = psB.tile([1, P], FP32, tag="ps_small")
        nc.tensor.transpose(pcm[:], colmin[:], ident[:])
        minall = work.tile([1, 2], FP32, tag="minall")
        nc.vector.tensor_reduce(minall[:, 0:1], pcm[:], axis=AX,
                                op=mybir.AluOpType.min)
        emit_fused(prev_stage, [2])

        for g in range(2):
            for dh in range(DH):
                pc2 = psA.tile([P, 512], FP32, tag="ps_mm", name=f"pc2{g}{dh}")
                nc.tensor.matmul(
                    pc2[:], qst16[:, dh * P:(dh + 1) * P],
                    PT16[:, g * 512:(g + 1) * 512],
                    start=True, stop=True,
                )
                nc.vector.tensor_mul(cxc16[:, dh, g * 512:(g + 1) * 512],
                                     ctxT16[:, dh, g * 512:(g + 1) * 512],
                                     pc2[:])
        pmb = psB.tile([P, 1], FP32, tag="ps_small")
        nc.tensor.matmul(pmb[:], ones_row[:], minall[:, 0:1], start=True,
                         stop=True)
        minb = work.tile([P, 1], FP32, tag="minb")
        nc.vector.tensor_copy(minb[:], pmb[:])
        wall16 = work.tile([P, CT], FP16, tag="wall16")  # exp(m - Mglob)
        nc.scalar.activation(wall16[:], madj[:], AF.Exp, bias=minb[:])
        emit_fused(prev_stage, [3, 4])

        # ---- q2c numerator/denominator + W_eff = A + D*diag(q2c) ----
        pn = psB.tile([1, D + 1], FP32, tag="ps_small", name="pn")
        for t in range(CT):
            nc.tensor.matmul(pn[:], wall16[:, t:t + 1], cnat16[:, t, 0:D + 1],
                             start=(t == 0), stop=(t == CT - 1))
        rden = work.tile([1, 1], FP32, tag="rden")
        nc.vector.reciprocal(rden[:], pn[0:1, D:D + 1])
        q2cr = work.tile([1, D], FP32, tag="q2cr")
        nc.vector.tensor_scalar_mul(q2cr[:], pn[0:1, 0:D], rden[:])
        emit_fused(prev_stage, [5])
        q2cc = work.tile([P, DH], FP32, tag="q2cc")  # [d_l, dh]
        for dh in range(DH):
            pq2 = psB.tile([P, 1], FP32, tag="ps_small", name=f"pq2{dh}")
            nc.tensor.transpose(pq2[:], q2cr[0:1, dh * P:(dh + 1) * P],
                                ident[0:1, 0:1])
            nc.vector.tensor_copy(q2cc[:, dh:dh + 1], pq2[:])
        weff16 = work.tile([P, DH, F], FP16, tag="weff16", bufs=2,
                           name=f"weff16_{b}")
        # Pool engine: slow but latency-tolerant mid-pipeline (not needed
        # until this elem's fused layer, a full stage away).  At the pipeline
        # ends (first/last elem) there is less PE fill, so use the faster DVE.
        weng = nc.vector if (pend is None or prev_stage is None) else nc.gpsimd
        for k in range(DH):
            weng.tensor_scalar_mul(weff16[:, k, :], w12t16[:, 6 + k, :],
                                   q2cc[:, k:k + 1])
            weng.tensor_add(weff16[:, k, :], weff16[:, k, :],
                            w12t16[:, k, :])

        stage = (b, ctxT16, cxc16, PT16, QB16, weff16)
        emit_fused(prev_stage, range(6, CT))  # finish elem b-1
        prev_stage = stage

    emit_fused(prev_stage, range(CT))  # drain: last elem's fused layer


_NC_CACHE = {}


def _build_nc(n_elems=BPC, reps=1):
    key = (n_elems, reps)
    if key in _NC_CACHE:
        return _NC_CACHE[key]
    nc = bacc.Bacc("TRN2", target_bir_lowering=False, debug=False,
                   num_devices=NCORES)
    ins = [
        nc.dram_tensor("ctxT", (n_elems, D, C), FP16, kind="ExternalInput").ap(),
        nc.dram_tensor("cnat", (n_elems, C, D + 1), FP16,
                       kind="ExternalInput").ap(),
        nc.dram_tensor("qst", (n_elems, Q, D), FP16, kind="ExternalInput").ap(),
        nc.dram_tensor("qmx", (n_elems, D, QP), FP16,
                       kind="ExternalInput").ap(),
        nc.dram_tensor("qwq", (n_elems, 1, QP), FP16,
                       kind="ExternalInput").ap(),
        nc.dram_tensor("qb", (n_elems, Q, F), FP16, kind="ExternalInput").ap(),
        nc.dram_tensor("w12t", (F, F), FP16, kind="ExternalInput").ap(),
        nc.dram_tensor("mT", (P, n_elems * CT), FP32, kind="ExternalInput").ap(),
    ]
    outs = [nc.dram_tensor("out", (n_elems, C, F), FP16,
                           kind="ExternalOutput").ap()]
    from contextlib import ExitStack
    with tile.TileContext(nc) as tc, ExitStack() as es:
        _build_body(es, tc, outs, ins, n_elems=n_elems, reps=reps)
    nc.compile()
    _NC_CACHE[key] = (nc, ins, outs)
    return _NC_CACHE[key]


def _host_prep(context, question, context_mask, w_question, w_context,
               w_multiple, W1, b1, W2, b2):
    """Input-only transforms: fp16 casts, transposes, q*wm scaling, the qwq
    row, QB' = question@B.T + b12, and the fused weights W12 = W2@W1."""
    context = np.asarray(context, np.float32)
    question = np.asarray(question, np.float32)
    maskf = np.asarray(context_mask).astype(np.float32)
    wq = np.asarray(w_question, np.float32)
    wc = np.asarray(w_context, np.float32)
    wm = np.asarray(w_multiple, np.float32)
    W1f = np.asarray(W1, np.float32)
    W2f = np.asarray(W2, np.float32)
    W12 = W2f @ W1f
    b12 = W2f @ np.asarray(b1, np.float32) + np.asarray(b2, np.float32)
    w12t16 = np.ascontiguousarray(W12.T.astype(np.float16))  # [f, f']

    ctx16 = context.astype(np.float16)
    ctxT16 = np.ascontiguousarray(ctx16.transpose(0, 2, 1))  # (B, D, C)
    cnat16 = np.concatenate(
        [ctx16, np.ones((B, C, 1), np.float16)], axis=2)     # (B, C, D+1)
    qst16 = question.astype(np.float16)                      # (B, Q, D)
    qmx16 = np.zeros((B, D, QP), np.float16)
    qmx16[:, :, 0:Q] = (question * wm).transpose(0, 2, 1)    # (q*wm)^T
    qmx16[:, :, Q] = wc.astype(np.float16)                   # cwc column
    qwq16 = np.zeros((B, 1, QP), np.float16)
    qwq16[:, 0, 0:Q] = question @ wq                         # qwq row
    qb16 = (question @ W12[:, D:2 * D].T + b12).astype(np.float16)  # QB'

    in_maps = []
    for i in range(NCORES):
        sl = slice(BPC * i, BPC * (i + 1))
        mTc = np.ascontiguousarray(
            maskf[sl].reshape(BPC, CT, P).transpose(2, 0, 1).reshape(P, BPC * CT))
        in_maps.append({
            "ctxT": np.ascontiguousarray(ctxT16[sl]),
            "cnat": np.ascontiguousarray(cnat16[sl]),
            "qst": np.ascontiguousarray(qst16[sl]),
            "qmx": np.ascontiguousarray(qmx16[sl]),
            "qwq": np.ascontiguousarray(qwq16[sl]),
            "qb": np.ascontiguousarray(qb16[sl]),
            "w12t": w12t16,
            "mT": mTc,
        })
    return in_maps


def kernel(context, question, context_mask, w_question, w_context, w_multiple,
           W1, b1, W2, b2):
    nc, _, _ = _build_nc()
    in_maps = _host_prep(context, question, context_mask, w_question,
                         w_context, w_multiple, W1, b1, W2, b2)
    res = run_bass_kernel_spmd(nc, in_maps, list(range(NCORES))).results
    out = np.concatenate([res[i]["out"] for i in range(NCORES)], axis=0)
    return out.astype(np.float32)


# revision 28
# speedup vs baseline: 1.0664x; 1.0664x over previous
"""Trainium2 Bass kernel for BaseBidirectionalAttention (fused-linear version).

Problem shapes (hardcoded): B=32, C=1024, Q=128, D=256, F=4D=1024.
Sharding: data-parallel over batch across 8 cores (4 batch elems/core);
weights replicated.

Algebraic restructurings vs the reference (all exact in real arithmetic):
  1. Fused linears: masking is row-wise and there is no nonlinearity between
     the two linears, so
       relu(((att@W1.T+b1)*m @ W2.T + b2)*m) = relu((att@W12.T + b12)*m)
     with W12 = W2@W1, b12 = W2@b1 + b2 precomputed on host.  Halves the
     dominant matmul work.
  2. att = [ctx, c2q, ctx*c2q, ctx*q2c]; q2c is constant over context rows,
     so the ctx and ctx*q2c pieces merge via a per-elem scaled weight block
     W_eff = A + D*diag(q2c)  (Pool-engine prep, no extra matmul k-steps).
  3. c2q = P @ question (P = softmax(sim) over q), so
     c2q @ B.T = P @ (question @ B.T) = P @ QB  -- and QB' = QB + b12 is
     input-only, precomputed on HOST (softmax rows sum to 1, so the bias
     rides along the P matmul for free).
  4. cwc = ctx.w_context folds into the sim matmul as a 129th moving column;
     qwq = question.w_question folds in as a K=1 accumulating matmul row.
  5. All input-only layout work (ctx/question transposes, fp16 casts,
     q*wm scaling, qwq row, QB') happens on host in _host_prep -- the device
     spends its cycles only on data-dependent compute.

Per-core per-elem device program (natural-layout output):
  sim(C,129)  = ctxT16.T @ [q*wm | w_c]  (+ qwq via K=1 row)      PE fp16
  P(C,Q)      = softmax_q(sim[:, :128])                           DVE/ACT
  PT(Q,C) transposes, cxc=(ctx*c2q)^T, W_eff=A+D*diag(q2c)
  out(C,F)    = relu((ctx@W_eff.T + cxc.T@C.T + P@QB') * m)       PE fp16

Everything heavy runs fp16 on the PE (1 cyc/row, hideable LDWEIGHTS+FWL);
softmax statistics, exp, q2c weighting and PSUM accumulation stay fp32.
Output is stored fp16 (halves the dominant DMA stream), upcast on host.
The per-elem fused layer is software-pipelined one stage behind the
attention phase: its matmuls are emitted interleaved into the next elem's
attention so PE never idles during the serial softmax/q2c chains.
"""

import sys

if "/opt/trn_rl_repo" not in sys.path:
    sys.path.insert(0, "/opt/trn_rl_repo")

import numpy as np

import concourse.bass as bass
import concourse.mybir as mybir
import concourse.tile as tile
from concourse import bacc
from concourse.bass_utils import run_bass_kernel_spmd
from concourse.masks import make_identity

B, C, Q, D = 32, 1024, 128, 256
F = 4 * D
NCORES = 8
BPC = B // NCORES  # batch elems per core
P = 128
CT = C // P   # 8 c-tiles
FT = F // P   # 8 f-tiles
DH = D // P   # 2 halves of D
QP = Q + 8    # padded moving-dim for the sim rhs (alignment)

FP32 = mybir.dt.float32
FP16 = mybir.dt.float16
AX = mybir.AxisListType.X
AF = mybir.ActivationFunctionType


def _build_body(es, tc, outs, ins, n_elems=BPC, reps=1):
    nc = tc.nc
    ctxT_d, cnat_d, qst_d, qmx_d, qwq_d, qb_d, w12t_d, mT_d = ins
    out_d = outs[0]

    const = es.enter_context(tc.tile_pool(name="const", bufs=1))
    weights = es.enter_context(tc.tile_pool(name="weights", bufs=1))
    loads = es.enter_context(tc.tile_pool(name="loads", bufs=3))
    work = es.enter_context(tc.tile_pool(name="work", bufs=1))
    outp = es.enter_context(tc.tile_pool(name="outp", bufs=4))
    psA = es.enter_context(tc.tile_pool(name="psA", bufs=5, space="PSUM"))
    psB = es.enter_context(tc.tile_pool(name="psB", bufs=3, space="PSUM"))

    # ---- constants / replicated weights ----
    ident = const.tile([P, P], FP32)
    make_identity(nc, ident)
    ident16 = const.tile([P, P], FP16)
    make_identity(nc, ident16)
    ones_row = const.tile([1, P], FP32)
    nc.vector.memset(ones_row, 1.0)
    ones16 = const.tile([1, P], FP16)
    nc.vector.memset(ones16, 1.0)

    def load_elem(b, idx):
        ctxT16 = loads.tile([P, DH, C], FP16, tag="ctxT16", name=f"ctxT{idx}")
        src = ctxT_d[b].rearrange("(h p) c -> p h c", p=P)
        nc.sync.dma_start(ctxT16[:, 0, :], src[:, 0])
        nc.sync.dma_start(ctxT16[:, 1, :], src[:, 1])
        cnat16 = loads.tile([P, CT, D + 4], FP16, tag="cnat16",
                            name=f"cnat{idx}")
        csrc = cnat_d[b].rearrange("(t p) d -> p t d", p=P)
        half = CT // 2
        nc.sync.dma_start(cnat16[:, :half, 0:D + 1], csrc[:, :half])
        nc.sync.dma_start(cnat16[:, half:, 0:D + 1], csrc[:, half:])
        qst16 = loads.tile([P, D], FP16, tag="qst16", name=f"qst{idx}")
        nc.sync.dma_start(qst16[:], qst_d[b])
        qmx16 = loads.tile([P, DH, QP], FP16, tag="qmx16", name=f"qmx{idx}")
        nc.sync.dma_start(qmx16[:], qmx_d[b].rearrange("(h p) j -> p h j", p=P))
        qwqx16 = loads.tile([1, QP], FP16, tag="qwqx16", name=f"qwq{idx}")
        nc.sync.dma_start(qwqx16[:], qwq_d[b])
        QB16 = loads.tile([P, F], FP16, tag="QB16", name=f"QB{idx}")
        nc.sync.dma_start(QB16[:], qb_d[b])
        return ctxT16, cnat16, qst16, qmx16, qwqx16, QB16

    # elem-0 loads go before the big weight DMA (single-shot only: with a
    # For_i timing loop the hoisted tile's slot would be recycled in-loop)
    pend = load_elem(0, 0) if reps == 1 else None

    w12t16 = weights.tile([P, FT, F], FP16)  # [fl, k, f'] = W12[f', k*128+fl]
    nc.sync.dma_start(w12t16[:], w12t_d.rearrange("(k p) f -> p k f", p=P))
    mT = const.tile([P, n_elems * CT], FP32)  # [p, b*8+t] = mask[b, t*128+p]
    nc.sync.dma_start(mT[:], mT_d)

    if reps > 1:
        es.enter_context(tc.For_i(0, reps, 1))

    def emit_fused(stage, cts):
        """Fused layer for `stage`'s elem (natural layout) + relu*mask +
        store.  Emitted interleaved into the NEXT elem's attention phase so
        its matmuls fill PE during that elem's serial q2c chain.  Piece
        order puts the late-arriving W_eff operands last."""
        if stage is None:
            return
        fb, fctxT16, fcxc16, fPT16, fQB16, fweff16 = stage
        for ct in cts:
            sl = slice(ct * P, (ct + 1) * P)
            pieces = [
                (fcxc16[:, 0, sl], w12t16[:, 4, :]),
                (fcxc16[:, 1, sl], w12t16[:, 5, :]),
                (fPT16[:, sl], fQB16[:]),
                (fctxT16[:, 0, sl], fweff16[:, 0, :]),
                (fctxT16[:, 1, sl], fweff16[:, 1, :]),
            ]
            p2 = [psA.tile([P, 512], FP32, tag="ps_mm", name=f"p2_{fb}{ct}{fh}")
                  for fh in range(2)]
            npc = len(pieces)
            for i, (lh, rh) in enumerate(pieces):
                for fh in range(2):
                    nc.tensor.matmul(
                        p2[fh][:], lh, rh[:, fh * 512:(fh + 1) * 512],
                        start=(i == 0), stop=(i == npc - 1),
                    )
            osb = outp.tile([P, F], FP16, tag="osb")
            mcol = mT[:, fb * CT + ct:fb * CT + ct + 1]
            # alternate evict engines so neither FIFO gates the PSUM ring;
            # odd groups go all-ACT to keep DVE clear for the softmax chain
            nc.scalar.activation(osb[:, 0:512], p2[0][:], AF.Relu, scale=mcol)
            if ct % 2 == 0:
                nc.vector.tensor_scalar(osb[:, 512:1024], p2[1][:], 0.0, mcol,
                                        op0=mybir.AluOpType.max,
                                        op1=mybir.AluOpType.mult)
            else:
                nc.scalar.activation(osb[:, 512:1024], p2[1][:], AF.Relu,
                                     scale=mcol)
            nc.sync.dma_start(out_d[fb, sl, :], osb[:])

    prev_stage = None
    for b in range(n_elems):
        # ---- loads (elem b prefetched; prefetch b+1 now) ----
        if pend is None:
            pend = load_elem(b, b)
        ctxT16, cnat16, qst16, qmx16, qwqx16, QB16 = pend
        pend = load_elem(b + 1, b + 1) if b + 1 < n_elems else None

        # ---- sim tiles + softmax over q (free dim), fused(b-1) groups
        # interleaved so PE never waits on the softmax consumers ----
        nmx = work.tile([P, CT], FP32, tag="nmx")    # negated row max
        sume = work.tile([P, CT], FP32, tag="sume")
        rs = work.tile([P, CT], FP32, tag="rs")
        pcwc = work.tile([P, CT], FP32, tag="pcwc")  # cwc columns [c_l, t]
        Pm16 = work.tile([P, CT, Q], FP16, tag="Pm16")

        def sim_tile(t):
            ps = psB.tile([P, Q + 1], FP32, tag="ps_small")
            for dh in range(DH):
                nc.tensor.matmul(
                    ps[:], ctxT16[:, dh, t * P:(t + 1) * P],
                    qmx16[:, dh, 0:Q + 1],
                    start=(dh == 0), stop=False,
                )
            nc.tensor.matmul(ps[:], ones16[:], qwqx16[0:1, 0:Q + 1],
                             start=False, stop=True)
            nc.vector.reduce_max(nmx[:, t:t + 1], ps[:, 0:Q], axis=AX,
                                 negate=True)
            nc.scalar.activation(
                Pm16[:, t, :], ps[:, 0:Q], AF.Exp, bias=nmx[:, t:t + 1],
                accum_out=sume[:, t:t + 1],
            )
            nc.vector.tensor_copy(pcwc[:, t:t + 1], ps[:, Q:Q + 1])
            nc.vector.reciprocal(rs[:, t:t + 1], sume[:, t:t + 1])
            nc.vector.tensor_scalar_mul(Pm16[:, t, :], Pm16[:, t, :],
                                        rs[:, t:t + 1])

        for t in range(4):
            sim_tile(t)
        emit_fused(prev_stage, [0])
        for t in range(4, CT):
            sim_tile(t)
        emit_fused(prev_stage, [1])

        # ---- q2c chain (DVE/ACT) with PT/c2q/fused as PE fill ----
        madj = work.tile([P, CT], FP32, tag="madj")  # m_c = cwc - nmx
        nc.vector.tensor_sub(madj[:], pcwc[:], nmx[:])
        colmin = work.tile([P, 1], FP32, tag="colmin")
        nc.vector.reduce_max(colmin[:], madj[:], axis=AX, negate=True)

        PT16 = work.tile([P, C], FP16, tag="PT16", bufs=2, name=f"PT16_{b}")
        cxc16 = work.tile([P, DH, C], FP16, tag="cxc16", bufs=2,
                          name=f"cxc16_{b}")
        for g in range(2):
            pt16 = psA.tile([P, 512], FP16, tag="ps_mm", name=f"pt16{g}")
            for j in range(4):
                t = g * 4 + j
                nc.tensor.transpose(pt16[:, j * P:(j + 1) * P], Pm16[:, t, :],
                                    ident16[:])
            nc.scalar.copy(PT16[:, g * 512:(g + 1) * 512], pt16[:])
        pcm = psB.tile([1, P], FP32, tag="ps_small")
        nc.tensor.transpose(pcm[:], colmin[:], ident[:])
        minall = work.tile([1, 2], FP32, tag="minall")
        nc.vector.tensor_reduce(minall[:, 0:1], pcm[:], axis=AX,
                                op=mybir.AluOpType.min)
        emit_fused(prev_stage, [2])

        for g in range(2):
            for dh in range(DH):
                pc2 = psA.tile([P, 512], FP32, tag="ps_mm", name=f"pc2{g}{dh}")
                nc.tensor.matmul(
                    pc2[:], qst16[:, dh * P:(dh + 1) * P],
                    PT16[:, g * 512:(g + 1) * 512],
                    start=True, stop=True,
                )
                nc.vector.tensor_mul(cxc16[:, dh, g * 512:(g + 1) * 512],
                                     ctxT16[:, dh, g * 512:(g + 1) * 512],
                                     pc2[:])
        pmb = psB.tile([P, 1], FP32, tag="ps_small")
        nc.tensor.matmul(pmb[:], ones_row[:], minall[:, 0:1], start=True,
                         stop=True)
        minb = work.tile([P, 1], FP32, tag="minb")
        nc.vector.tensor_copy(minb[:], pmb[:])
        wall16 = work.tile([P, CT], FP16, tag="wall16")  # exp(m - Mglob)
        nc.scalar.activation(wall16[:], madj[:], AF.Exp, bias=minb[:])
        emit_fused(prev_stage, [3, 4])

        # ---- q2c numerator/denominator + W_eff = A + D*diag(q2c) ----
        pn = psB.tile([1, D + 1], FP32, tag="ps_small", name="pn")
        for t in range(CT):
            nc.tensor.matmul(pn[:], wall16[:, t:t + 1], cnat16[:, t, 0:D + 1],
                             start=(t == 0), stop=(t == CT - 1))
        rden = work.tile([1, 1], FP32, tag="rden")
        nc.vector.reciprocal(rden[:], pn[0:1, D:D + 1])
        q2cr = work.tile([1, D], FP32, tag="q2cr")
        nc.vector.tensor_scalar_mul(q2cr[:], pn[0:1, 0:D], rden[:])
        emit_fused(prev_stage, [5])
        q2cc = work.tile([P, DH], FP32, tag="q2cc")  # [d_l, dh]
        for dh in range(DH):
            pq2 = psB.tile([P, 1], FP32, tag="ps_small", name=f"pq2{dh}")
            nc.tensor.transpose(pq2[:], q2cr[0:1, dh * P:(dh + 1) * P],
                                ident[0:1, 0:1])
            nc.vector.tensor_copy(q2cc[:, dh:dh + 1], pq2[:])
        weff16 = work.tile([P, DH, F], FP16, tag="weff16", bufs=2,
                           name=f"weff16_{b}")
        # Pool engine: slow but latency-tolerant mid-pipeline (not needed
        # until this elem's fused layer, a full stage away).  At the pipeline
        # ends (first/last elem) there is less PE fill, so use the faster DVE.
        weng = nc.vector if (pend is None or prev_stage is None) else nc.gpsimd
        for k in range(DH):
            weng.tensor_scalar_mul(weff16[:, k, :], w12t16[:, 6 + k, :],
                                   q2cc[:, k:k + 1])
            weng.tensor_add(weff16[:, k, :], weff16[:, k, :],
                            w12t16[:, k, :])

        stage = (b, ctxT16, cxc16, PT16, QB16, weff16)
        emit_fused(prev_stage, range(6, CT))  # finish elem b-1
        prev_stage = stage

    emit_fused(prev_stage, range(CT))  # drain: last elem's fused layer


_NC_CACHE = {}


def _build_nc(n_elems=BPC, reps=1):
    key = (n_elems, reps)
    if key in _NC_CACHE:
        return _NC_CACHE[key]
    nc = bacc.Bacc("TRN2", target_bir_lowering=False, debug=False,
                   num_devices=NCORES)
    ins = [
        nc.dram_tensor("ctxT", (n_elems, D, C), FP16, kind="ExternalInput").ap(),
        nc.dram_tensor("cnat", (n_elems, C, D + 1), FP16,
                       kind="ExternalInput").ap(),
        nc.dram_tensor("qst", (n_elems, Q, D), FP16, kind="ExternalInput").ap(),
        nc.dram_tensor("qmx", (n_elems, D, QP), FP16,
                       kind="ExternalInput").ap(),
        nc.dram_tensor("qwq", (n_elems, 1, QP), FP16,
                       kind="ExternalInput").ap(),
        nc.dram_tensor("qb", (n_elems, Q, F), FP16, kind="ExternalInput").ap(),
        nc.dram_tensor("w12t", (F, F), FP16, kind="ExternalInput").ap(),
        nc.dram_tensor("mT", (P, n_elems * CT), FP32, kind="ExternalInput").ap(),
    ]
    outs = [nc.dram_tensor("out", (n_elems, C, F), FP16,
                           kind="ExternalOutput").ap()]
    from contextlib import ExitStack
    with tile.TileContext(nc) as tc, ExitStack() as es:
        _build_body(es, tc, outs, ins, n_elems=n_elems, reps=reps)
    nc.compile()
    _NC_CACHE[key] = (nc, ins, outs)
    return _NC_CACHE[key]


def _host_prep(context, question, context_mask, w_question, w_context,
               w_multiple, W1, b1, W2, b2):
    """Input-only transforms: fp16 casts, transposes, q*wm scaling, the qwq
    row, QB' = question@B.T + b12, and the fused weights W12 = W2@W1."""
    context = np.asarray(context, np.float32)
    question = np.asarray(question, np.float32)
    maskf = np.asarray(context_mask).astype(np.float32)
    wq = np.asarray(w_question, np.float32)
    wc = np.asarray(w_context, np.float32)
    wm = np.asarray(w_multiple, np.float32)
    W1f = np.asarray(W1, np.float32)
    W2f = np.asarray(W2, np.float32)
    W12 = W2f @ W1f
    b12 = W2f @ np.asarray(b1, np.float32) + np.asarray(b2, np.float32)
    w12t16 = np.ascontiguousarray(W12.T.astype(np.float16))  # [f, f']

    ctx16 = context.astype(np.float16)
    ctxT16 = np.ascontiguousarray(ctx16.transpose(0, 2, 1))  # (B, D, C)
    cnat16 = np.concatenate(
        [ctx16, np.ones((B, C, 1), np.float16)], axis=2)     # (B, C, D+1)
    qst16 = question.astype(np.float16)                      # (B, Q, D)
    qmx16 = np.zeros((B, D, QP), np.float16)
    qmx16[:, :, 0:Q] = (question * wm).transpose(0, 2, 1)    # (q*wm)^T
    qmx16[:, :, Q] = wc.astype(np.float16)                   # cwc column
    qwq16 = np.zeros((B, 1, QP), np.float16)
    qwq16[:, 0, 0:Q] = question @ wq                         # qwq row
    qb16 = (question @ W12[:, D:2 * D].T + b12).astype(np.float16)  # QB'

    in_maps = []
    for i in range(NCORES):
        sl = slice(BPC * i, BPC * (i + 1))
        mTc = np.ascontiguousarray(
            maskf[sl].reshape(BPC, CT, P).transpose(2, 0, 1).reshape(P, BPC * CT))
        in_maps.append({
            "ctxT": np.ascontiguousarray(ctxT16[sl]),
            "cnat": np.ascontiguousarray(cnat16[sl]),
            "qst": np.ascontiguousarray(qst16[sl]),
            "qmx": np.ascontiguousarray(qmx16[sl]),
            "qwq": np.ascontiguousarray(qwq16[sl]),
            "qb": np.ascontiguousarray(qb16[sl]),
            "w12t": w12t16,
            "mT": mTc,
        })
    return in_maps


def kernel(context, question, context_mask, w_question, w_context, w_multiple,
           W1, b1, W2, b2):
    nc, _, _ = _build_nc()
    in_maps = _host_prep(context, question, context_mask, w_question,
                         w_context, w_multiple, W1, b1, W2, b2)
    res = run_bass_kernel_spmd(nc, in_maps, list(range(NCORES))).results
    out = np.concatenate([res[i]["out"] for i in range(NCORES)], axis=0)
    return out.astype(np.float32)


# revision 30
# speedup vs baseline: 1.3361x; 1.2528x over previous
"""Trainium2 Bass kernel for BaseBidirectionalAttention (fused-linear version).

Problem shapes (hardcoded): B=32, C=1024, Q=128, D=256, F=4D=1024.
Sharding: data-parallel over batch across 8 cores (4 batch elems/core);
weights replicated.

Algebraic restructurings vs the reference (all exact in real arithmetic):
  1. Fused linears: masking is row-wise and there is no nonlinearity between
     the two linears, so
       relu(((att@W1.T+b1)*m @ W2.T + b2)*m) = relu((att@W12.T + b12)*m)
     with W12 = W2@W1, b12 = W2@b1 + b2 precomputed on host.  Halves the
     dominant matmul work.
  2. att = [ctx, c2q, ctx*c2q, ctx*q2c]; q2c is constant over context rows,
     so the ctx and ctx*q2c pieces merge via a per-elem scaled weight block
     W_eff = A + D*diag(q2c)  (Pool-engine prep, no extra matmul k-steps).
  3. c2q = P @ question (P = softmax(sim) over q), so
     c2q @ B.T = P @ (question @ B.T) = P @ QB  -- and QB' = QB + b12 is
     input-only, precomputed on HOST (softmax rows sum to 1, so the bias
     rides along the P matmul for free).
  4. cwc = ctx.w_context folds into the sim matmul as a 129th moving column;
     qwq = question.w_question folds in as a K=1 accumulating matmul row.
  5. All input-only layout work (ctx/question transposes, fp16 casts,
     q*wm scaling, qwq row, QB') happens on host in _host_prep -- the device
     spends its cycles only on data-dependent compute.

Per-core per-elem device program (natural-layout output):
  sim(C,129)  = ctxT16.T @ [q*wm | w_c]  (+ qwq via K=1 row)      PE fp16
  P(C,Q)      = softmax_q(sim[:, :128])                           DVE/ACT
  PT(Q,C) transposes, cxc=(ctx*c2q)^T, W_eff=A+D*diag(q2c)
  out(C,F)    = relu((ctx@W_eff.T + cxc.T@C.T + P@QB') * m)       PE fp16

Everything heavy runs fp16 on the PE (1 cyc/row, hideable LDWEIGHTS+FWL);
softmax statistics, exp, q2c weighting and PSUM accumulation stay fp32.
Output is stored fp16 (halves the dominant DMA stream), upcast on host.
The per-elem fused layer is software-pipelined one stage behind the
attention phase: its matmuls are emitted interleaved into the next elem's
attention so PE never idles during the serial softmax/q2c chains.
"""

import sys

if "/opt/trn_rl_repo" not in sys.path:
    sys.path.insert(0, "/opt/trn_rl_repo")

import numpy as np

import concourse.bass as bass
import concourse.mybir as mybir
import concourse.tile as tile
from concourse import bacc
from concourse.bass_utils import run_bass_kernel_spmd
from concourse.masks import make_identity

B, C, Q, D = 32, 1024, 128, 256
F = 4 * D
NCORES = 8
BPC = B // NCORES  # batch elems per core
P = 128
CT = C // P   # 8 c-tiles
FT = F // P   # 8 f-tiles
DH = D // P   # 2 halves of D
QP = Q + 8    # padded moving-dim for the sim rhs (alignment)

FP32 = mybir.dt.float32
FP16 = mybir.dt.float16
AX = mybir.AxisListType.X
AF = mybir.ActivationFunctionType


def _build_body(es, tc, outs, ins, n_elems=BPC, reps=1):
    nc = tc.nc
    ctxT_d, cnat_d, qst_d, qmx_d, qwq_d, qb_d, w12t_d, mT_d = ins
    out_d = outs[0]

    const = es.enter_context(tc.tile_pool(name="const", bufs=1))
    weights = es.enter_context(tc.tile_pool(name="weights", bufs=1))
    loads = es.enter_context(tc.tile_pool(name="loads", bufs=3))
    work = es.enter_context(tc.tile_pool(name="work", bufs=1))
    outp = es.enter_context(tc.tile_pool(name="outp", bufs=4))
    psA = es.enter_context(tc.tile_pool(name="psA", bufs=5, space="PSUM"))
    psB = es.enter_context(tc.tile_pool(name="psB", bufs=3, space="PSUM"))

    # ---- constants / replicated weights ----
    ident = const.tile([P, P], FP32)
    make_identity(nc, ident)
    ident16 = const.tile([P, P], FP16)
    make_identity(nc, ident16)
    ones_row = const.tile([1, P], FP32)
    nc.vector.memset(ones_row, 1.0)
    ones16 = const.tile([1, P], FP16)
    nc.vector.memset(ones16, 1.0)

    def load_elem(b, idx):
        ctxT16 = loads.tile([P, DH, C], FP16, tag="ctxT16", name=f"ctxT{idx}")
        src = ctxT_d[b].rearrange("(h p) c -> p h c", p=P)
        nc.sync.dma_start(ctxT16[:, 0, :], src[:, 0])
        nc.sync.dma_start(ctxT16[:, 1, :], src[:, 1])
        cnat16 = loads.tile([P, CT, D + 4], FP16, tag="cnat16",
                            name=f"cnat{idx}")
        csrc = cnat_d[b].rearrange("(t p) d -> p t d", p=P)
        half = CT // 2
        nc.sync.dma_start(cnat16[:, :half, 0:D + 1], csrc[:, :half])
        nc.sync.dma_start(cnat16[:, half:, 0:D + 1], csrc[:, half:])
        qst16 = loads.tile([P, D], FP16, tag="qst16", name=f"qst{idx}")
        nc.sync.dma_start(qst16[:], qst_d[b])
        qmx16 = loads.tile([P, DH, QP], FP16, tag="qmx16", name=f"qmx{idx}")
        nc.sync.dma_start(qmx16[:], qmx_d[b].rearrange("(h p) j -> p h j", p=P))
        qwqx16 = loads.tile([1, QP], FP16, tag="qwqx16", name=f"qwq{idx}")
        nc.sync.dma_start(qwqx16[:], qwq_d[b])
        QB16 = loads.tile([P, F], FP16, tag="QB16", name=f"QB{idx}")
        nc.sync.dma_start(QB16[:], qb_d[b])
        return ctxT16, cnat16, qst16, qmx16, qwqx16, QB16

    # elem-0 loads go before the big weight DMA (single-shot only: with a
    # For_i timing loop the hoisted tile's slot would be recycled in-loop)
    pend = load_elem(0, 0) if reps == 1 else None

    w12t16 = weights.tile([P, FT, F], FP16)  # [fl, k, f'] = W12[f', k*128+fl]
    nc.sync.dma_start(w12t16[:], w12t_d.rearrange("(k p) f -> p k f", p=P))
    mT = const.tile([P, n_elems * CT], FP32)  # [p, b*8+t] = mask[b, t*128+p]
    nc.sync.dma_start(mT[:], mT_d)

    if reps > 1:
        es.enter_context(tc.For_i(0, reps, 1))

    def emit_fused(stage, cts):
        """Fused layer for `stage`'s elem (natural layout) + relu*mask +
        store.  Emitted interleaved into the NEXT elem's attention phase so
        its matmuls fill PE during that elem's serial q2c chain.  Piece
        order puts the late-arriving W_eff operands last."""
        if stage is None:
            return
        fb, fctxT16, fcxc16, fPT16, fQB16, fweff16 = stage
        for ct in cts:
            sl = slice(ct * P, (ct + 1) * P)
            pieces = [
                (fcxc16[:, 0, sl], w12t16[:, 4, :]),
                (fcxc16[:, 1, sl], w12t16[:, 5, :]),
                (fPT16[:, sl], fQB16[:]),
                (fctxT16[:, 0, sl], fweff16[:, 0, :]),
                (fctxT16[:, 1, sl], fweff16[:, 1, :]),
            ]
            p2 = [psA.tile([P, 512], FP32, tag="ps_mm", name=f"p2_{fb}{ct}{fh}")
                  for fh in range(2)]
            npc = len(pieces)
            for i, (lh, rh) in enumerate(pieces):
                for fh in range(2):
                    nc.tensor.matmul(
                        p2[fh][:], lh, rh[:, fh * 512:(fh + 1) * 512],
                        start=(i == 0), stop=(i == npc - 1),
                    )
            osb = outp.tile([P, F], FP16, tag="osb")
            mcol = mT[:, fb * CT + ct:fb * CT + ct + 1]
            # alternate evict engines so neither FIFO gates the PSUM ring;
            # odd groups go all-ACT to keep DVE clear for the softmax chain
            nc.scalar.activation(osb[:, 0:512], p2[0][:], AF.Relu, scale=mcol)
            if ct % 2 == 0:
                nc.vector.tensor_scalar(osb[:, 512:1024], p2[1][:], 0.0, mcol,
                                        op0=mybir.AluOpType.max,
                                        op1=mybir.AluOpType.mult)
            else:
                nc.scalar.activation(osb[:, 512:1024], p2[1][:], AF.Relu,
                                     scale=mcol)
            nc.sync.dma_start(out_d[fb, sl, :], osb[:])

    prev_stage = None
    for b in range(n_elems):
        # ---- loads (elem b prefetched; prefetch b+1 now) ----
        if pend is None:
            pend = load_elem(b, b)
        ctxT16, cnat16, qst16, qmx16, qwqx16, QB16 = pend
        pend = load_elem(b + 1, b + 1) if b + 1 < n_elems else None

        # ---- sim tiles + softmax over q (free dim), fused(b-1) groups
        # interleaved so PE never waits on the softmax consumers ----
        nmx = work.tile([P, CT], FP32, tag="nmx")    # negated row max
        sume = work.tile([P, CT], FP32, tag="sume")
        rs = work.tile([P, CT], FP32, tag="rs")
        pcwc = work.tile([P, CT], FP32, tag="pcwc")  # cwc columns [c_l, t]
        Pm16 = work.tile([P, CT, Q], FP16, tag="Pm16")

        def sim_tile(t):
            ps = psB.tile([P, Q + 1], FP32, tag="ps_small")
            for dh in range(DH):
                nc.tensor.matmul(
                    ps[:], ctxT16[:, dh, t * P:(t + 1) * P],
                    qmx16[:, dh, 0:Q + 1],
                    start=(dh == 0), stop=False,
                )
            nc.tensor.matmul(ps[:], ones16[:], qwqx16[0:1, 0:Q + 1],
                             start=False, stop=True)
            nc.vector.reduce_max(nmx[:, t:t + 1], ps[:, 0:Q], axis=AX,
                                 negate=True)
            nc.scalar.activation(
                Pm16[:, t, :], ps[:, 0:Q], AF.Exp, bias=nmx[:, t:t + 1],
                accum_out=sume[:, t:t + 1],
            )
            nc.vector.tensor_copy(pcwc[:, t:t + 1], ps[:, Q:Q + 1])
            nc.vector.reciprocal(rs[:, t:t + 1], sume[:, t:t + 1])
            nc.vector.tensor_scalar_mul(Pm16[:, t, :], Pm16[:, t, :],
                                        rs[:, t:t + 1])

        for t in range(4):
            sim_tile(t)
        emit_fused(prev_stage, [0])
        for t in range(4, CT):
            sim_tile(t)
        emit_fused(prev_stage, [1])

        # ---- q2c chain (DVE/ACT) with PT/c2q/fused as PE fill ----
        madj = work.tile([P, CT], FP32, tag="madj")  # m_c = cwc - nmx
        nc.vector.tensor_sub(madj[:], pcwc[:], nmx[:])
        colmin = work.tile([P, 1], FP32, tag="colmin")
        nc.vector.reduce_max(colmin[:], madj[:], axis=AX, negate=True)

        PT16 = work.tile([P, C], FP16, tag="PT16", bufs=2, name=f"PT16_{b}")
        cxc16 = work.tile([P, DH, C], FP16, tag="cxc16", bufs=2,
                          name=f"cxc16_{b}")
        for g in range(2):
            pt16 = psA.tile([P, 512], FP16, tag="ps_mm", name=f"pt16{g}")
            for j in range(4):
                t = g * 4 + j
                nc.tensor.transpose(pt16[:, j * P:(j + 1) * P], Pm16[:, t, :],
                                    ident16[:])
            nc.scalar.copy(PT16[:, g * 512:(g + 1) * 512], pt16[:])
        pcm = psB.tile([1, P], FP32, tag="ps_small")
        nc.tensor.transpose(pcm[:], colmin[:], ident[:])
        minall = work.tile([1, 2], FP32, tag="minall")
        nc.vector.tensor_reduce(minall[:, 0:1], pcm[:], axis=AX,
                                op=mybir.AluOpType.min)
        emit_fused(prev_stage, [2])

        for g in range(2):
            for dh in range(DH):
                pc2 = psA.tile([P, 512], FP32, tag="ps_mm", name=f"pc2{g}{dh}")
                nc.tensor.matmul(
                    pc2[:], qst16[:, dh * P:(dh + 1) * P],
                    PT16[:, g * 512:(g + 1) * 512],
                    start=True, stop=True,
                )
                nc.vector.tensor_mul(cxc16[:, dh, g * 512:(g + 1) * 512],
                                     ctxT16[:, dh, g * 512:(g + 1) * 512],
                                     pc2[:])
        pmb = psB.tile([P, 1], FP32, tag="ps_small")
        nc.tensor.matmul(pmb[:], ones_row[:], minall[:, 0:1], start=True,
                         stop=True)
        minb = work.tile([P, 1], FP32, tag="minb")
        nc.vector.tensor_copy(minb[:], pmb[:])
        wall16 = work.tile([P, CT], FP16, tag="wall16")  # exp(m - Mglob)
        nc.scalar.activation(wall16[:], madj[:], AF.Exp, bias=minb[:])
        emit_fused(prev_stage, [3, 4])

        # ---- q2c numerator/denominator + W_eff = A + D*diag(q2c) ----
        pn = psB.tile([1, D + 1], FP32, tag="ps_small", name="pn")
        for t in range(CT):
            nc.tensor.matmul(pn[:], wall16[:, t:t + 1], cnat16[:, t, 0:D + 1],
                             start=(t == 0), stop=(t == CT - 1))
        rden = work.tile([1, 1], FP32, tag="rden")
        nc.vector.reciprocal(rden[:], pn[0:1, D:D + 1])
        q2cr = work.tile([1, D], FP32, tag="q2cr")
        nc.vector.tensor_scalar_mul(q2cr[:], pn[0:1, 0:D], rden[:])
        emit_fused(prev_stage, [5])
        q2cc = work.tile([P, DH], FP32, tag="q2cc")  # [d_l, dh]
        for dh in range(DH):
            pq2 = psB.tile([P, 1], FP32, tag="ps_small", name=f"pq2{dh}")
            nc.tensor.transpose(pq2[:], q2cr[0:1, dh * P:(dh + 1) * P],
                                ident[0:1, 0:1])
            nc.vector.tensor_copy(q2cc[:, dh:dh + 1], pq2[:])
        weff16 = work.tile([P, DH, F], FP16, tag="weff16", bufs=2,
                           name=f"weff16_{b}")
        # Pool engine: slow but latency-tolerant mid-pipeline (not needed
        # until this elem's fused layer, a full stage away).  At the pipeline
        # ends (first/last elem) there is less PE fill, so use the faster DVE.
        weng = nc.vector if (pend is None or prev_stage is None) else nc.gpsimd
        for k in range(DH):
            weng.tensor_scalar_mul(weff16[:, k, :], w12t16[:, 6 + k, :],
                                   q2cc[:, k:k + 1])
            weng.tensor_add(weff16[:, k, :], weff16[:, k, :],
                            w12t16[:, k, :])

        stage = (b, ctxT16, cxc16, PT16, QB16, weff16)
        emit_fused(prev_stage, range(6, CT))  # finish elem b-1
        prev_stage = stage

    emit_fused(prev_stage, range(CT))  # drain: last elem's fused layer


_NC_CACHE = {}


def _build_nc(n_elems=BPC, reps=1):
    key = (n_elems, reps)
    if key in _NC_CACHE:
        return _NC_CACHE[key]
    nc = bacc.Bacc("TRN2", target_bir_lowering=False, debug=False,
                   num_devices=NCORES)
    ins = [
        nc.dram_tensor("ctxT", (n_elems, D, C), FP16, kind="ExternalInput").ap(),
        nc.dram_tensor("cnat", (n_elems, C, D + 1), FP16,
                       kind="ExternalInput").ap(),
        nc.dram_tensor("qst", (n_elems, Q, D), FP16, kind="ExternalInput").ap(),
        nc.dram_tensor("qmx", (n_elems, D, QP), FP16,
                       kind="ExternalInput").ap(),
        nc.dram_tensor("qwq", (n_elems, 1, QP), FP16,
                       kind="ExternalInput").ap(),
        nc.dram_tensor("qb", (n_elems, Q, F), FP16, kind="ExternalInput").ap(),
        nc.dram_tensor("w12t", (F, F), FP16, kind="ExternalInput").ap(),
        nc.dram_tensor("mT", (P, n_elems * CT), FP32, kind="ExternalInput").ap(),
    ]
    outs = [nc.dram_tensor("out", (n_elems, C, F), FP16,
                           kind="ExternalOutput").ap()]
    from contextlib import ExitStack
    with tile.TileContext(nc) as tc, ExitStack() as es:
        _build_body(es, tc, outs, ins, n_elems=n_elems, reps=reps)
    nc.compile()
    _NC_CACHE[key] = (nc, ins, outs)
    return _NC_CACHE[key]


def _host_prep(context, question, context_mask, w_question, w_context,
               w_multiple, W1, b1, W2, b2):
    """Input-only transforms: fp16 casts, transposes, q*wm scaling, the qwq
    row, QB' = question@B.T + b12, and the fused weights W12 = W2@W1."""
    context = np.asarray(context, np.float32)
    question = np.asarray(question, np.float32)
    maskf = np.asarray(context_mask).astype(np.float32)
    wq = np.asarray(w_question, np.float32)
    wc = np.asarray(w_context, np.float32)
    wm = np.asarray(w_multiple, np.float32)
    W1f = np.asarray(W1, np.float32)
    W2f = np.asarray(W2, np.float32)
    W12 = W2f @ W1f
    b12 = W2f @ np.asarray(b1, np.float32) + np.asarray(b2, np.float32)
    w12t16 = np.ascontiguousarray(W12.T.astype(np.float16))  # [f, f']

    ctx16 = context.astype(np.float16)
    ctxT16 = np.ascontiguousarray(ctx16.transpose(0, 2, 1))  # (B, D, C)
    cnat16 = np.concatenate(
        [ctx16, np.ones((B, C, 1), np.float16)], axis=2)     # (B, C, D+1)
    qst16 = question.astype(np.float16)                      # (B, Q, D)
    qmx16 = np.zeros((B, D, QP), np.float16)
    qmx16[:, :, 0:Q] = (question * wm).transpose(0, 2, 1)    # (q*wm)^T
    qmx16[:, :, Q] = wc.astype(np.float16)                   # cwc column
    qwq16 = np.zeros((B, 1, QP), np.float16)
    qwq16[:, 0, 0:Q] = question @ wq                         # qwq row
    qb16 = (question @ W12[:, D:2 * D].T + b12).astype(np.float16)  # QB'

    in_maps = []
    for i in range(NCORES):
        sl = slice(BPC * i, BPC * (i + 1))
        mTc = np.ascontiguousarray(
            maskf[sl].reshape(BPC, CT, P).transpose(2, 0, 1).reshape(P, BPC * CT))
        in_maps.append({
            "ctxT": np.ascontiguousarray(ctxT16[sl]),
            "cnat": np.ascontiguousarray(cnat16[sl]),
            "qst": np.ascontiguousarray(qst16[sl]),
            "qmx": np.ascontiguousarray(qmx16[sl]),
            "qwq": np.ascontiguousarray(qwq16[sl]),
            "qb": np.ascontiguousarray(qb16[sl]),
            "w12t": w12t16,
            "mT": mTc,
        })
    return in_maps


def kernel(context, question, context_mask, w_question, w_context, w_multiple,
           W1, b1, W2, b2):
    nc, _, _ = _build_nc()
    in_maps = _host_prep(context, question, context_mask, w_question,
                         w_context, w_multiple, W1, b1, W2, b2)
    res = run_bass_kernel_spmd(nc, in_maps, list(range(NCORES))).results
    out = np.concatenate([res[i]["out"] for i in range(NCORES)], axis=0)
    return out.astype(np.float32)
